# revision 1
# baseline (speedup 1.0000x reference)
"""Trainium2 Bass kernel for nn_BayesianNN (attention + bayesian NEAT scan).

Strategy (8 NeuronCores, SPMD):
  - Shard Wq/Wk/Wv rows (output features) across cores; each core computes
    Q^T/K^T shards = W_shard @ X^T via PE (bf16 tiles, on-chip PE transpose
    of the weight tiles, fp32 PSUM accumulation).
  - Partial S = Q^T(shard)^T @ K^T(shard) accumulated in PSUM, AllReduce'd
    ([256,256] fp32) -> softmax + column-mean a_bar computed redundantly.
  - V path needs only context = (a_bar @ X) @ Wv^T: computed as a fp32
    dot-product reduction (DVE tensor_tensor_reduce) over natural-layout Wv
    tiles against a replicated u = a_bar @ X. No transpose of Wv needed.
  - base = context @ W[:D, D:] computed shard-wise on natural-layout slab
    tiles (mu + sigma*eps fused on DVE), AllReduce'd ([260] fp32).
  - 260-step topological scan on-device: ACT tanh + rank-1 PE matmuls
    (v_k * A[k, k+1:]) accumulating into a PSUM row; A in bf16 with hi/lo
    split of v for near-fp32 accuracy. Scan runs redundantly on all cores.
"""
import sys

for _p in ("/opt/trn_rl_repo",):
    if _p not in sys.path:
        sys.path.insert(0, _p)

import numpy as np

M = 256
D = 7686
HO = 260
NTOT = D + HO
NCORES = 8
SH = 1024  # padded shard rows per core
SCALE = float(1.0 / np.sqrt(np.float32(D)))

# row shard boundaries (cores 0..6 get 961 rows, core 7 gets 959)
SIZES = [961] * 7 + [959]
STARTS = [sum(SIZES[:c]) for c in range(NCORES)]

# d-axis macro chunks for streaming weight loads
D_MACROS = [(i * 1024, 1024) for i in range(7)] + [(7168, 518)]
# d-axis 512-chunks for u
D_CH512 = [(i * 512, 512) for i in range(15)] + [(7680, 6)]

_CACHE = {}
import os
KPHASE = os.environ.get("KPHASE", "full")


def _build():
    import concourse.mybir as mybir
    import concourse.tile as tile
    from concourse import bacc
    from concourse.masks import make_identity
    from contextlib import ExitStack

    dt = mybir.dt
    f32, bf = dt.float32, dt.bfloat16
    AF = mybir.ActivationFunctionType
    ALU = mybir.AluOpType
    AX = mybir.AxisListType

    nc = bacc.Bacc(None, target_bir_lowering=False, num_devices=NCORES)

    X = nc.dram_tensor("x", [M, D], f32, kind="ExternalInput")
    Wq = nc.dram_tensor("wq", [SH, D], f32, kind="ExternalInput")
    Wk = nc.dram_tensor("wk", [SH, D], f32, kind="ExternalInput")
    Wv = nc.dram_tensor("wv", [SH, D], f32, kind="ExternalInput")
    BQ = nc.dram_tensor("bq", [SH], f32, kind="ExternalInput")
    BK = nc.dram_tensor("bk", [SH], f32, kind="ExternalInput")
    BV = nc.dram_tensor("bv", [SH], f32, kind="ExternalInput")
    MUS = nc.dram_tensor("mus", [SH, HO], f32, kind="ExternalInput")
    SGS = nc.dram_tensor("sgs", [SH, HO], f32, kind="ExternalInput")
    EPS = nc.dram_tensor("eps", [SH, HO], f32, kind="ExternalInput")
    MUA = nc.dram_tensor("mua", [HO, HO], f32, kind="ExternalInput")
    SGA = nc.dram_tensor("sga", [HO, HO], f32, kind="ExternalInput")
    EPA = nc.dram_tensor("epa", [HO, HO], f32, kind="ExternalInput")
    BMU = nc.dram_tensor("bmu", [HO], f32, kind="ExternalInput")
    BSG = nc.dram_tensor("bsg", [HO], f32, kind="ExternalInput")
    EPB = nc.dram_tensor("epb", [HO], f32, kind="ExternalInput")
    Y = nc.dram_tensor("y", [4], f32, kind="ExternalOutput")

    RG = [list(range(NCORES))]

    with tile.TileContext(nc) as tc, ExitStack() as ctx:
        # long-lived small pools (~30 KB/partition total)
        const = ctx.enter_context(tc.tile_pool(name="const", bufs=1))
        sm = ctx.enter_context(tc.tile_pool(name="sm", bufs=1))
        ctxpool = ctx.enter_context(tc.tile_pool(name="ctxpool", bufs=1))
        dram = ctx.enter_context(tc.tile_pool(name="dram", bufs=1, space="DRAM"))
        umid_cm = tc.tile_pool(name="umid", bufs=1)
        upool = umid_cm.__enter__()

        idb = const.tile([128, 128], bf, tag="idb")
        make_identity(nc, idb)
        ones_f = const.tile([128, 1], f32, tag="ones_f")
        nc.vector.memset(ones_f[:], 1.0)
        ones_b = const.tile([1, 128], bf, tag="ones_b")
        nc.vector.memset(ones_b[:], 1.0)

        u_bf = upool.tile([1, D], bf, tag="u_bf")
        u_f32 = upool.tile([1, D], f32, tag="u_f32")
        u_lo = upool.tile([1, D], bf, tag="u_lo")
        ctx_sb = ctxpool.tile([128, 8], f32, tag="ctx_sb")

        with tc.tile_pool(name="pa_big", bufs=1) as pab, \
             tc.tile_pool(name="wload", bufs=6) as wload, \
             tc.tile_pool(name="wcast", bufs=4) as wcast, \
             tc.tile_pool(name="wtp", bufs=6) as wtp, \
             tc.tile_pool(name="qk", bufs=4) as qk:

            # ---------- phase 0: X load (streamed), bf16 cast, X^T build ----------
            xb = pab.tile([128, 2, D], bf, tag="xb")
            for h in range(2):
                for (mst, msz) in D_MACROS:
                    xl = wload.tile([128, 1024], f32, tag="wl")
                    nc.sync.dma_start(
                        xl[:, :msz],
                        X[h * 128:(h + 1) * 128, mst:mst + msz])
                    nc.scalar.copy(out=xb[:, h, mst:mst + msz], in_=xl[:, :msz])

            # xt[d_rel, dc, m] = X[m, dc*128 + d_rel] in bf16; d-chunk 60 is
            # the 6-wide tail, zero padded.
            xt = pab.tile([128, 61, 256], bf, tag="xt")
            nc.vector.memset(xt[:, 60, :], 0.0)

            with tc.tile_pool(name="pt", bufs=4, space="PSUM") as ptp, \
                 tc.tile_pool(name="pqt", bufs=3, space="PSUM") as pqtp, \
                 tc.tile_pool(name="ps", bufs=1, space="PSUM") as psp:
                for h in range(2):
                    for c in range(61):
                        w = 128 if c < 60 else 6
                        pt = ptp.tile([128, 128], bf, tag="pt")
                        nc.tensor.transpose(pt[:w, :], xb[:, h, c * 128:c * 128 + w], idb[:])
                        nc.vector.tensor_copy(out=xt[:w, c, h * 128:(h + 1) * 128], in_=pt[:w, :])

                # biases for Q,K as [128, 8] per-partition layout
                bq_sb = sm.tile([128, 8], f32, tag="bq_sb")
                nc.sync.dma_start(bq_sb[:], BQ[:].rearrange("(c p) -> p c", p=128))
                bk_sb = sm.tile([128, 8], f32, tag="bk_sb")
                nc.sync.dma_start(bk_sb[:], BK[:].rearrange("(c p) -> p c", p=128))

                # ---------- phase 1: Q^T/K^T shard matmuls + partial S ----------
                s_ps = psp.tile([128, 2, 256], f32, tag="s_ps")
                for ic in range(8):
                    qt_tiles = {}
                    for mat, (wsrc, bias_sb) in (("q", (Wq, bq_sb)), ("k", (Wk, bk_sb))):
                        acc = pqtp.tile([128, 256], f32, tag="pqt")
                        dc = 0
                        for im, (mst, msz) in enumerate(D_MACROS):
                            wl = wload.tile([128, 1024], f32, tag="wl")
                            nc.sync.dma_start(wl[:, :msz], wsrc[ic * 128:(ic + 1) * 128, mst:mst + msz])
                            wc = wcast.tile([128, 1024], bf, tag="wc")
                            nc.scalar.copy(out=wc[:, :msz], in_=wl[:, :msz])
                            nin = (msz + 127) // 128
                            for c in range(nin):
                                w = min(128, msz - c * 128)
                                pt = ptp.tile([128, 128], bf, tag="pt")
                                nc.tensor.transpose(pt[:w, :], wc[:, c * 128:c * 128 + w], idb[:])
                                wt_sb = wtp.tile([128, 128], bf, tag="wt")
                                nc.vector.tensor_copy(out=wt_sb[:w, :], in_=pt[:w, :])
                                nc.tensor.matmul(acc[:], lhsT=wt_sb[:w, :], rhs=xt[:w, dc, :],
                                                 start=(dc == 0), stop=(dc == 60))
                                dc += 1
                        qt = qk.tile([128, 256], bf, tag="qt")
                        nc.scalar.activation(out=qt[:], in_=acc[:], func=AF.Identity,
                                             bias=bias_sb[:, ic:ic + 1], scale=1.0)
                        qt_tiles[mat] = qt
                    for h in range(2):
                        # s_ps halves share one PSUM bank (zero region):
                        # only the first MM starts the group, only the last stops it.
                        nc.tensor.matmul(s_ps[:, h, :], lhsT=qt_tiles["q"][:, h * 128:(h + 1) * 128],
                                         rhs=qt_tiles["k"][:], start=(ic == 0 and h == 0),
                                         stop=(ic == 7 and h == 1))

                # ---------- phase 2a: AllReduce S ----------
                s_in = dram.tile([M, M], f32, tag="s_in")
                s_out = dram.tile([M, M], f32, tag="s_out", addr_space="Shared")
                s_sb = sm.tile([128, 2, 256], f32, tag="s_sb")
                nc.scalar.copy(out=s_sb[:], in_=s_ps[:])
                nc.sync.dma_start(s_in[:].rearrange("(h p) f -> p h f", p=128), s_sb[:])
                nc.gpsimd.collective_compute("AllReduce", ALU.add, replica_groups=RG,
                                             ins=[s_in[:].opt()], outs=[s_out[:].opt()])

            sr = sm.tile([128, 2, 256], f32, tag="sr")
            nc.sync.dma_start(sr[:], s_out[:].rearrange("(h p) f -> p h f", p=128))

            # ---------- phase 2b: softmax rows + a_bar + u ----------
            ex = sm.tile([128, 2, 256], f32, tag="ex")
            mx = sm.tile([128, 2], f32, tag="mx")
            nm = sm.tile([128, 2], f32, tag="nm")
            rs = sm.tile([128, 2], f32, tag="rs")
            inv = sm.tile([128, 2], f32, tag="inv")
            for h in range(2):
                nc.vector.tensor_reduce(mx[:, h:h + 1], sr[:, h, :], axis=AX.X, op=ALU.max)
                nc.vector.tensor_scalar_mul(nm[:, h:h + 1], mx[:, h:h + 1], -SCALE)
                nc.scalar.activation(out=ex[:, h, :], in_=sr[:, h, :], func=AF.Exp,
                                     bias=nm[:, h:h + 1], scale=SCALE,
                                     accum_out=rs[:, h:h + 1])
                nc.vector.reciprocal(inv[:, h:h + 1], rs[:, h:h + 1])
                nc.vector.tensor_scalar_mul(ex[:, h, :], ex[:, h, :], inv[:, h:h + 1])

            with tc.tile_pool(name="psm", bufs=3, space="PSUM") as psmp:
                # a_barT[mc*128+q] = sum_m attn[m, mc*128+q] (then /M)
                abt_ps = psmp.tile([128, 2], f32, tag="abt")
                for mc in range(2):
                    for h in range(2):
                        nc.tensor.matmul(abt_ps[:, mc:mc + 1], lhsT=ex[:, h, mc * 128:(mc + 1) * 128],
                                         rhs=ones_f[:], start=(h == 0), stop=(h == 1))
                abf = sm.tile([128, 2], f32, tag="abf_sb")
                nc.scalar.mul(out=abf[:], in_=abt_ps[:], mul=1.0 / M)
                abt = sm.tile([128, 2], bf, tag="abt_sb")
                nc.scalar.copy(out=abt[:], in_=abf[:])
                ablo = sm.tile([128, 2], bf, tag="ablo_sb")
                nc.vector.tensor_sub(ablo[:], abf[:], abt[:])

                # u = a_bar @ X  (bf16 matmul with hi/lo a_bar, fp32 psum)
                for (st, sz) in D_CH512:
                    up = psmp.tile([1, 512], f32, tag="up")
                    for i, (ab_part, mc) in enumerate(
                            [(abt, 0), (abt, 1), (ablo, 0), (ablo, 1)]):
                        nc.tensor.matmul(up[0:1, :sz], lhsT=ab_part[:, mc:mc + 1],
                                         rhs=xb[:, mc, st:st + sz],
                                         start=(i == 0), stop=(i == 3))
                    nc.scalar.copy(out=u_f32[0:1, st:st + sz], in_=up[0:1, :sz])
                    nc.scalar.copy(out=u_bf[0:1, st:st + sz], in_=up[0:1, :sz])
                    nc.vector.tensor_sub(u_lo[0:1, st:st + sz], u_f32[0:1, st:st + sz],
                                         u_bf[0:1, st:st + sz])

        # pa_big closed: xb/xt space released
        with tc.tile_pool(name="pb_big", bufs=1) as pbb, \
             tc.tile_pool(name="wvp", bufs=10) as wvp, \
             tc.tile_pool(name="psr", bufs=3, space="PSUM") as psrp:
            # replicate u to all 128 partitions (fp32)
            u_rep = pbb.tile([128, D], f32, tag="u_rep")
            for (st, sz) in D_CH512:
                rp = psrp.tile([128, 512], f32, tag="rp")
                nc.tensor.matmul(rp[:, :sz], lhsT=ones_b[:], rhs=u_bf[0:1, st:st + sz],
                                 start=True, stop=False)
                nc.tensor.matmul(rp[:, :sz], lhsT=ones_b[:], rhs=u_lo[0:1, st:st + sz],
                                 start=False, stop=True)
                nc.vector.tensor_copy(out=u_rep[:, st:st + sz], in_=rp[:, :sz])

            # ---------- phase 3: context shard via fp32 dot reductions ----------
            nmac = len(D_MACROS)
            ctxp = pbb.tile([128, 8, nmac], f32, tag="ctxp")
            for ic in range(8):
                for im, (mst, msz) in enumerate(D_MACROS):
                    wv_t = wvp.tile([128, 1024], f32, tag="wv")
                    nc.sync.dma_start(wv_t[:, :msz], Wv[ic * 128:(ic + 1) * 128, mst:mst + msz])
                    prod = wvp.tile([128, 1024], f32, tag="prod")
                    nc.vector.scalar_tensor_tensor(
                        out=prod[:, :msz], in0=wv_t[:, :msz], scalar=1.0,
                        in1=u_rep[:, mst:mst + msz], op0=ALU.mult, op1=ALU.mult,
                        accum_out=ctxp[:, ic, im:im + 1])
            bv_sb = sm.tile([128, 8], f32, tag="bv_sb")
            nc.sync.dma_start(bv_sb[:], BV[:].rearrange("(c p) -> p c", p=128))
            for ic in range(8):
                nc.vector.tensor_reduce(ctx_sb[:, ic:ic + 1], ctxp[:, ic, :], axis=AX.X, op=ALU.add)
            nc.vector.tensor_add(ctx_sb[:], ctx_sb[:], bv_sb[:])

        umid_cm.__exit__(None, None, None)

        # ---------- phase 4: partial base = ctx_shard @ slab, AllReduce ----------
        with tc.tile_pool(name="slabp", bufs=2) as slabp, \
             tc.tile_pool(name="pbase", bufs=1, space="PSUM") as pbp:
            base_ps = pbp.tile([1, HO], f32, tag="base_ps")
            for ic in range(8):
                m_t = slabp.tile([128, HO], f32, tag="smu")
                s_t = slabp.tile([128, HO], f32, tag="ssg")
                e_t = slabp.tile([128, HO], f32, tag="sep")
                nc.sync.dma_start(m_t[:], MUS[ic * 128:(ic + 1) * 128, :])
                nc.sync.dma_start(s_t[:], SGS[ic * 128:(ic + 1) * 128, :])
                nc.sync.dma_start(e_t[:], EPS[ic * 128:(ic + 1) * 128, :])
                nc.vector.tensor_mul(s_t[:], s_t[:], e_t[:])
                nc.vector.tensor_add(m_t[:], m_t[:], s_t[:])
                nc.tensor.matmul(base_ps[0:1, :], lhsT=ctx_sb[:, ic:ic + 1], rhs=m_t[:],
                                 start=(ic == 0), stop=(ic == 7))
            pb_sb = sm.tile([1, HO], f32, tag="pb_sb")
            nc.vector.tensor_copy(out=pb_sb[:], in_=base_ps[:])
        b_in = dram.tile([1, HO], f32, tag="b_in")
        b_out = dram.tile([1, HO], f32, tag="b_out", addr_space="Shared")
        nc.sync.dma_start(b_in[:], pb_sb[:])
        nc.gpsimd.collective_compute("AllReduce", ALU.add, replica_groups=RG,
                                     ins=[b_in[:].opt()], outs=[b_out[:].opt()])

        if KPHASE == "noscan":
            yb = sm.tile([1, HO], f32, tag="yb_dbg")
            nc.sync.dma_start(yb[:], b_out[:])
            nc.sync.dma_start(Y[:], yb[0:1, 0:4])

        # ---------- phase 5+6: A block prep + sequential 260-step scan ----------
        with tc.tile_pool(name="scanp", bufs=1) as scanp:
          if KPHASE != "noscan":
            aA = scanp.tile([128, 3, HO], f32, tag="aA")
            sA = scanp.tile([128, 3, HO], f32, tag="sA")
            eA = scanp.tile([128, 3, HO], f32, tag="eA")
            nc.sync.dma_start(aA[:, 0:2, :], MUA[0:256, :].rearrange("(c p) f -> p c f", p=128))
            nc.sync.dma_start(aA[0:4, 2, :], MUA[256:260, :])
            nc.sync.dma_start(sA[:, 0:2, :], SGA[0:256, :].rearrange("(c p) f -> p c f", p=128))
            nc.sync.dma_start(sA[0:4, 2, :], SGA[256:260, :])
            nc.sync.dma_start(eA[:, 0:2, :], EPA[0:256, :].rearrange("(c p) f -> p c f", p=128))
            nc.sync.dma_start(eA[0:4, 2, :], EPA[256:260, :])
            nc.vector.tensor_mul(sA[:, 0:2, :], sA[:, 0:2, :], eA[:, 0:2, :])
            nc.vector.tensor_add(aA[:, 0:2, :], aA[:, 0:2, :], sA[:, 0:2, :])
            nc.vector.tensor_mul(sA[0:4, 2, :], sA[0:4, 2, :], eA[0:4, 2, :])
            nc.vector.tensor_add(aA[0:4, 2, :], aA[0:4, 2, :], sA[0:4, 2, :])
            ab = scanp.tile([128, 3, HO], bf, tag="ab")
            nc.scalar.copy(out=ab[:, 0:2, :], in_=aA[:, 0:2, :])
            nc.scalar.copy(out=ab[0:4, 2, :], in_=aA[0:4, 2, :])
            a_dram = dram.tile([HO, HO], bf, tag="a_dram")
            nc.sync.dma_start(a_dram[0:256, :].rearrange("(c p) f -> p c f", p=128), ab[:, 0:2, :])
            nc.sync.dma_start(a_dram[256:260, :], ab[0:4, 2, :])
            a_p0 = scanp.tile([1, HO * HO], bf, tag="a_p0")
            nc.sync.dma_start(a_p0[:], a_dram[:].rearrange("a b -> (a b)")[None, :])

            # base_full = AllReduce(partial) + (bias_mu + bias_sigma * eps_b)
            base_sb = scanp.tile([1, HO], f32, tag="base_sb")
            nc.sync.dma_start(base_sb[:], b_out[:])
            bb_m = sm.tile([1, HO], f32, tag="bb_m")
            bb_s = sm.tile([1, HO], f32, tag="bb_s")
            bb_e = sm.tile([1, HO], f32, tag="bb_e")
            nc.sync.dma_start(bb_m[:], BMU[:][None, :])
            nc.sync.dma_start(bb_s[:], BSG[:][None, :])
            nc.sync.dma_start(bb_e[:], EPB[:][None, :])
            nc.vector.tensor_mul(bb_s[:], bb_s[:], bb_e[:])
            nc.vector.tensor_add(base_sb[:], base_sb[:], bb_s[:])
            nc.vector.tensor_add(base_sb[:], base_sb[:], bb_m[:])

            vhi = scanp.tile([1, HO], bf, tag="vhi")
            vf = scanp.tile([1, HO], f32, tag="vf")
            vlo = scanp.tile([1, HO], bf, tag="vlo")
            with tc.tile_pool(name="pf", bufs=1, space="PSUM") as pfp:
                fs = pfp.tile([1, HO], f32, tag="fs")
                for k in range(HO):
                    # one fp32 tanh; bf16 hi value + lo correction derived from it
                    if k == 0:
                        nc.scalar.activation(out=vf[0:1, 0:1], in_=base_sb[0:1, 0:1], func=AF.Tanh)
                    else:
                        nc.scalar.activation(out=vf[0:1, k:k + 1], in_=fs[0:1, k:k + 1], func=AF.Tanh,
                                             bias=base_sb[0:1, k:k + 1], scale=1.0)
                    nc.scalar.copy(out=vhi[0:1, k:k + 1], in_=vf[0:1, k:k + 1])
                    nc.vector.tensor_sub(vlo[0:1, k:k + 1], vf[0:1, k:k + 1], vhi[0:1, k:k + 1])
                    if k < HO - 1:
                        nc.tensor.matmul(fs[0:1, k + 1:HO], lhsT=vhi[0:1, k:k + 1],
                                         rhs=a_p0[0:1, k * HO + k + 1:(k + 1) * HO],
                                         start=(k == 0), stop=(k == HO - 2),
                                         skip_group_check=True)
                    if k < HO - 2:
                        nc.tensor.matmul(fs[0:1, k + 2:HO], lhsT=vlo[0:1, k:k + 1],
                                         rhs=a_p0[0:1, k * HO + k + 2:(k + 1) * HO],
                                         start=False, stop=False, skip_group_check=True)
                nc.sync.dma_start(Y[:], vf[0:1, HO - 4:HO])

    nc.compile()
    return nc


def _get_nc():
    if "nc" not in _CACHE:
        _CACHE["nc"] = _build()
    return _CACHE["nc"]


def _make_in_maps(inputs):
    X = np.ascontiguousarray(np.asarray(inputs["input_matrix"], np.float32))
    Wq = np.asarray(inputs["Wq"], np.float32)
    Wk = np.asarray(inputs["Wk"], np.float32)
    Wv = np.asarray(inputs["Wv"], np.float32)
    bq = np.asarray(inputs["bq"], np.float32)
    bk = np.asarray(inputs["bk"], np.float32)
    bv = np.asarray(inputs["bv"], np.float32)
    mu = np.asarray(inputs["weight_mu"], np.float32)
    sg = np.asarray(inputs["weight_sigma"], np.float32)
    ep = np.asarray(inputs["eps_w"], np.float32)
    bmu = np.asarray(inputs["bias_mu"], np.float32)
    bsg = np.asarray(inputs["bias_sigma"], np.float32)
    epb = np.asarray(inputs["eps_b"], np.float32)

    muA = np.ascontiguousarray(mu[D:NTOT, D:NTOT])
    sgA = np.ascontiguousarray(sg[D:NTOT, D:NTOT])
    epA = np.ascontiguousarray(ep[D:NTOT, D:NTOT])

    in_maps = []
    for c in range(NCORES):
        st, sz = STARTS[c], SIZES[c]

        def rows2d(A):
            out = np.zeros((SH, A.shape[1]), np.float32)
            out[:sz] = A[st:st + sz]
            return out

        def rows1d(a):
            out = np.zeros((SH,), np.float32)
            out[:sz] = a[st:st + sz]
            return out

        in_maps.append({
            "x": X,
            "wq": rows2d(Wq), "wk": rows2d(Wk), "wv": rows2d(Wv),
            "bq": rows1d(bq), "bk": rows1d(bk), "bv": rows1d(bv),
            "mus": rows2d(mu[:, D:NTOT]),
            "sgs": rows2d(sg[:, D:NTOT]),
            "eps": rows2d(ep[:, D:NTOT]),
            "mua": muA, "sga": sgA, "epa": epA,
            "bmu": bmu, "bsg": bsg, "epb": epb,
        })
    return in_maps


def kernel(**inputs):
    from concourse.bass_utils import run_bass_kernel_spmd

    nc = _get_nc()
    in_maps = _make_in_maps(inputs)
    res = run_bass_kernel_spmd(nc, in_maps, core_ids=list(range(NCORES)))
    return np.asarray(res.results[0]["y"], np.float32).reshape(4)



# revision 10
# speedup vs baseline: 22547.1991x; 22547.1991x over previous
"""Trainium2 Bass kernel for nn_BayesianNN (attention + bayesian NEAT scan).

Strategy (8 NeuronCores, SPMD, feature/row tensor-parallel):
  - Shard Wq/Wk/Wv rows (output features) across cores. Per core, W tiles
    stream in fp32, are cast to bf16 (split between ACT and DVE), PE
    transposes them into PSUM banks (8 tiles/bank), one batched copy per
    bank returns them to SBUF, and PE accumulates Q^T/K^T/V^T shards
    = W_shard @ X^T in fp32 PSUM.
  - Partial S = Q^T(shard)^T @ K^T(shard) and partial P = V @ slab
    (slab = mu+sigma*eps rows of the input->hidden block) are both
    computed in phase 1 and AllReduce'd together in one [256,516] fp32
    collective. After softmax, base = (a_bar @ P) needs just two small
    matmuls - no second collective round-trip before the bias add.
  - 260-step topological scan: one tanh per node on ACT
    (v[j] = tanh(v[j-1]*A[j-1,j] + pre[j])), with each node's
    contributions to columns >= j+2 applied off-path by a DVE rank-1
    update into the fp32 pre-activation row. v stays fp32.
"""
import sys

for _p in ("/opt/trn_rl_repo",):
    if _p not in sys.path:
        sys.path.insert(0, _p)

import numpy as np

M = 256
D = 7686
DP = 7808          # D padded to 61*128 for clean 128-chunking
NCH = DP // 128    # 61 d-chunks
HO = 260
NTOT = D + HO
NCORES = 8
SH = 1024          # padded shard rows per core
SCALE = float(1.0 / np.sqrt(np.float32(D)))

SIZES = [961] * 7 + [959]
STARTS = [sum(SIZES[:c]) for c in range(NCORES)]

# d-axis macro chunks (8 chunks of 128 each except the last with 5)
MACROS = [(i * 8, 8) for i in range(7)] + [(56, 5)]  # (chunk0, nchunks)

_CACHE = {}


def _build():
    import concourse.mybir as mybir
    import concourse.tile as tile
    from concourse import bacc
    from concourse.masks import make_identity
    from contextlib import ExitStack

    dt = mybir.dt
    f32, bf = dt.float32, dt.bfloat16
    AF = mybir.ActivationFunctionType
    ALU = mybir.AluOpType
    AX = mybir.AxisListType

    nc = bacc.Bacc(None, target_bir_lowering=False, num_devices=NCORES)

    X = nc.dram_tensor("x", [M, DP], f32, kind="ExternalInput")
    Wq = nc.dram_tensor("wq", [SH, DP], f32, kind="ExternalInput")
    Wk = nc.dram_tensor("wk", [SH, DP], f32, kind="ExternalInput")
    Wv = nc.dram_tensor("wv", [SH, DP], f32, kind="ExternalInput")
    BQ = nc.dram_tensor("bq", [SH], f32, kind="ExternalInput")
    BK = nc.dram_tensor("bk", [SH], f32, kind="ExternalInput")
    BV = nc.dram_tensor("bv", [SH], f32, kind="ExternalInput")
    MUS = nc.dram_tensor("mus", [SH, HO], f32, kind="ExternalInput")
    SGS = nc.dram_tensor("sgs", [SH, HO], f32, kind="ExternalInput")
    EPS = nc.dram_tensor("eps", [SH, HO], f32, kind="ExternalInput")
    MUA = nc.dram_tensor("mua", [HO, HO], f32, kind="ExternalInput")
    SGA = nc.dram_tensor("sga", [HO, HO], f32, kind="ExternalInput")
    EPA = nc.dram_tensor("epa", [HO, HO], f32, kind="ExternalInput")
    BMU = nc.dram_tensor("bmu", [HO], f32, kind="ExternalInput")
    BSG = nc.dram_tensor("bsg", [HO], f32, kind="ExternalInput")
    EPB = nc.dram_tensor("epb", [HO], f32, kind="ExternalInput")
    Y = nc.dram_tensor("y", [4], f32, kind="ExternalOutput")

    RG = [list(range(NCORES))]

    with tile.TileContext(nc) as tc, ExitStack() as ctx:
        const = ctx.enter_context(tc.tile_pool(name="const", bufs=1))
        sm = ctx.enter_context(tc.tile_pool(name="sm", bufs=1))
        vtp = ctx.enter_context(tc.tile_pool(name="vtp", bufs=1))
        scanp = ctx.enter_context(tc.tile_pool(name="scanp", bufs=1))
        dram = ctx.enter_context(tc.tile_pool(name="dram", bufs=1, space="DRAM"))

        idb = const.tile([128, 128], bf, tag="idb")
        make_identity(nc, idb)
        ones_f = const.tile([128, 1], f32, tag="ones_f")
        nc.vector.memset(ones_f[:], 1.0)

        vt_sb = vtp.tile([128, 8, 256], f32, tag="vt_sb")
        slab_sb = vtp.tile([128, 8, HO], f32, tag="slab_sb")

        band = scanp.tile([1, 259, 1], f32, tag="band")
        vrow = scanp.tile([1, HO], f32, tag="vrow")
        pre_sb = scanp.tile([1, HO], f32, tag="pre_sb")
        bb_s = scanp.tile([1, HO], f32, tag="bb_s")
        y4 = scanp.tile([1, 4], f32, tag="y4")

        # ---------- early prep: A block combine + scan tables + slab ----------
        with tc.tile_pool(name="aprep", bufs=1) as aprep, \
             tc.tile_pool(name="slabl", bufs=2) as slabl:
            aA = aprep.tile([128, 3, HO], f32, tag="aA")
            sA = aprep.tile([128, 3, HO], f32, tag="sA")
            eA = aprep.tile([128, 3, HO], f32, tag="eA")
            nc.sync.dma_start(aA[:, 0:2, :], MUA[0:256, :].rearrange("(c p) f -> p c f", p=128))
            nc.sync.dma_start(aA[0:4, 2, :], MUA[256:260, :])
            nc.sync.dma_start(sA[:, 0:2, :], SGA[0:256, :].rearrange("(c p) f -> p c f", p=128))
            nc.sync.dma_start(sA[0:4, 2, :], SGA[256:260, :])
            nc.sync.dma_start(eA[:, 0:2, :], EPA[0:256, :].rearrange("(c p) f -> p c f", p=128))
            nc.sync.dma_start(eA[0:4, 2, :], EPA[256:260, :])
            nc.vector.tensor_mul(sA[:, 0:2, :], sA[:, 0:2, :], eA[:, 0:2, :])
            nc.vector.tensor_add(aA[:, 0:2, :], aA[:, 0:2, :], sA[:, 0:2, :])
            nc.vector.tensor_mul(sA[0:4, 2, :], sA[0:4, 2, :], eA[0:4, 2, :])
            nc.vector.tensor_add(aA[0:4, 2, :], aA[0:4, 2, :], sA[0:4, 2, :])
            ab = aprep.tile([128, 3, HO], bf, tag="ab")
            nc.scalar.copy(out=ab[:, 0:2, :], in_=aA[:, 0:2, :])
            nc.scalar.copy(out=ab[0:4, 2, :], in_=aA[0:4, 2, :])
            a_dram = dram.tile([HO, HO], bf, tag="a_dram")
            nc.sync.dma_start(a_dram[0:256, :].rearrange("(c p) f -> p c f", p=128), ab[:, 0:2, :])
            nc.sync.dma_start(a_dram[256:260, :], ab[0:4, 2, :])
            af_dram = dram.tile([HO, HO], f32, tag="af_dram")
            nc.sync.dma_start(af_dram[0:256, :].rearrange("(c p) f -> p c f", p=128), aA[:, 0:2, :])
            nc.sync.dma_start(af_dram[256:260, :], aA[0:4, 2, :])
            # superdiagonal: band[0, k, 0] = A[k, k+1]
            af_flat = af_dram[:].rearrange("a b -> (a b)")
            nc.sync.dma_start(
                band[:], af_flat[1:1 + 259 * 261].rearrange("(k s) -> k s", s=261)[None, :, 0:1])

            bb_m = aprep.tile([1, HO], f32, tag="bb_m")
            bb_e = aprep.tile([1, HO], f32, tag="bb_e")
            nc.sync.dma_start(bb_m[:], BMU[:][None, :])
            nc.sync.dma_start(bb_s[:], BSG[:][None, :])
            nc.sync.dma_start(bb_e[:], EPB[:][None, :])
            nc.vector.tensor_mul(bb_s[:], bb_s[:], bb_e[:])
            nc.vector.tensor_add(bb_s[:], bb_s[:], bb_m[:])

            for ic in range(8):
                m_t = slabl.tile([128, HO], f32, tag="smu")
                s_t = slabl.tile([128, HO], f32, tag="ssg")
                e_t = slabl.tile([128, HO], f32, tag="sep")
                nc.sync.dma_start(m_t[:], MUS[ic * 128:(ic + 1) * 128, :])
                nc.sync.dma_start(s_t[:], SGS[ic * 128:(ic + 1) * 128, :])
                nc.sync.dma_start(e_t[:], EPS[ic * 128:(ic + 1) * 128, :])
                nc.vector.tensor_mul(s_t[:], s_t[:], e_t[:])
                nc.vector.tensor_add(slab_sb[:, ic, :], m_t[:], s_t[:])

        # ---------- phase 0+1: X^T build, then QKV shard matmuls ----------
        with tc.tile_pool(name="pa_big", bufs=1) as pab, \
             tc.tile_pool(name="wload", bufs=4) as wload, \
             tc.tile_pool(name="wcast", bufs=4) as wcast, \
             tc.tile_pool(name="wtp", bufs=4) as wtp, \
             tc.tile_pool(name="qk", bufs=4) as qk:

            bq_sb = sm.tile([128, 8], f32, tag="bq_sb")
            nc.sync.dma_start(bq_sb[:], BQ[:].rearrange("(c p) -> p c", p=128))
            bk_sb = sm.tile([128, 8], f32, tag="bk_sb")
            nc.sync.dma_start(bk_sb[:], BK[:].rearrange("(c p) -> p c", p=128))
            bv_sb = sm.tile([128, 8], f32, tag="bv_sb")
            nc.sync.dma_start(bv_sb[:], BV[:].rearrange("(c p) -> p c", p=128))

            # xt[d%128, d//128, h*128+m] = X[h*128+m, d] in bf16
            xt = pab.tile([128, NCH, 256], bf, tag="xt")

            with tc.tile_pool(name="ptr", bufs=2, space="PSUM") as ptrp, \
                 tc.tile_pool(name="pacc", bufs=2, space="PSUM") as paccp, \
                 tc.tile_pool(name="ps", bufs=1, space="PSUM") as psp, \
                 tc.tile_pool(name="pp", bufs=1, space="PSUM") as ppp, \
                 tc.tile_pool(name="xbp", bufs=1) as xbp:

                # --- X: load, cast, transpose (batched copies) ---
                xb = xbp.tile([128, 2, DP], bf, tag="xb")
                for h in range(2):
                    for im, (c0, nch) in enumerate(MACROS):
                        xl = wload.tile([128, 1024], f32, tag="wl")
                        nc.sync.dma_start(xl[:, :nch * 128],
                                          X[h * 128:(h + 1) * 128, c0 * 128:(c0 + nch) * 128])
                        eng = nc.scalar if im % 2 == 0 else nc.vector
                        if eng is nc.scalar:
                            nc.scalar.copy(out=xb[:, h, c0 * 128:(c0 + nch) * 128],
                                           in_=xl[:, :nch * 128])
                        else:
                            nc.vector.tensor_copy(out=xb[:, h, c0 * 128:(c0 + nch) * 128],
                                                  in_=xl[:, :nch * 128])
                for h in range(2):
                    for (c0, nch) in MACROS:
                        ptr = ptrp.tile([128, 8, 128], bf, tag="ptr")
                        for c in range(nch):
                            nc.tensor.transpose(ptr[:, c, :], xb[:, h, (c0 + c) * 128:(c0 + c + 1) * 128], idb[:])
                        nc.vector.tensor_copy(out=xt[:, c0:c0 + nch, h * 128:(h + 1) * 128],
                                              in_=ptr[:, 0:nch, :])

                # --- QKV streaming ---
                s_ps = psp.tile([128, 2, 256], f32, tag="s_ps")
                # P partial: 512-wide halves so each [*, h, 0:260] slice is
                # bank-aligned (1040B used of each 2KB bank)
                p_ps = ppp.tile([128, 2, 512], f32, tag="p_ps")
                for ic in range(8):
                    qt_tiles = {}
                    for mat, (wsrc, bias_sb) in (
                            ("q", (Wq, bq_sb)), ("k", (Wk, bk_sb)), ("v", (Wv, bv_sb))):
                        acc = paccp.tile([128, 256], f32, tag="pacc")
                        for im, (c0, nch) in enumerate(MACROS):
                            wl = wload.tile([128, 1024], f32, tag="wl")
                            nc.sync.dma_start(wl[:, :nch * 128],
                                              wsrc[ic * 128:(ic + 1) * 128, c0 * 128:(c0 + nch) * 128])
                            wc = wcast.tile([128, 1024], bf, tag="wc")
                            if im % 3 == 0:
                                nc.vector.tensor_copy(out=wc[:, :nch * 128], in_=wl[:, :nch * 128])
                            else:
                                nc.scalar.copy(out=wc[:, :nch * 128], in_=wl[:, :nch * 128])
                            ptr = ptrp.tile([128, 8, 128], bf, tag="ptr")
                            for c in range(nch):
                                nc.tensor.transpose(ptr[:, c, :], wc[:, c * 128:(c + 1) * 128], idb[:])
                            wt = wtp.tile([128, 8, 128], bf, tag="wt")
                            nc.vector.tensor_copy(out=wt[:, 0:nch, :], in_=ptr[:, 0:nch, :])
                            for c in range(nch):
                                nc.tensor.matmul(acc[:], lhsT=wt[:, c, :], rhs=xt[:, c0 + c, :],
                                                 start=(c0 + c == 0), stop=(c0 + c == NCH - 1))
                        if mat == "v":
                            nc.scalar.activation(out=vt_sb[:, ic, :], in_=acc[:],
                                                 func=AF.Identity,
                                                 bias=bias_sb[:, ic:ic + 1], scale=1.0)
                        else:
                            qt = qk.tile([128, 256], bf, tag="qt")
                            nc.scalar.activation(out=qt[:], in_=acc[:], func=AF.Identity,
                                                 bias=bias_sb[:, ic:ic + 1], scale=1.0)
                            qt_tiles[mat] = qt
                    for h in range(2):
                        nc.tensor.matmul(s_ps[:, h, :], lhsT=qt_tiles["q"][:, h * 128:(h + 1) * 128],
                                         rhs=qt_tiles["k"][:], start=(ic == 0 and h == 0),
                                         stop=(ic == 7 and h == 1))
                        nc.tensor.matmul(p_ps[:, h, 0:HO], lhsT=vt_sb[:, ic, h * 128:(h + 1) * 128],
                                         rhs=slab_sb[:, ic, :], start=(ic == 0),
                                         stop=(ic == 7), skip_group_check=True)

                # ---------- phase 2a: fused AllReduce of [S | P] ----------
                sp_in = dram.tile([M, 516], f32, tag="sp_in")
                sp_out = dram.tile([M, 516], f32, tag="sp_out", addr_space="Shared")
                s_sb = sm.tile([128, 2, 256], f32, tag="s_sb")
                nc.scalar.copy(out=s_sb[:], in_=s_ps[:])
                p_sb = sm.tile([128, 2, HO], f32, tag="p_sb")
                nc.scalar.copy(out=p_sb[:], in_=p_ps[:, :, 0:HO])
                nc.sync.dma_start(sp_in[:, 0:256].rearrange("(h p) f -> p h f", p=128), s_sb[:])
                nc.sync.dma_start(sp_in[:, 256:516].rearrange("(h p) f -> p h f", p=128), p_sb[:])
                nc.gpsimd.collective_compute("AllReduce", ALU.add, replica_groups=RG,
                                             ins=[sp_in[:].opt()], outs=[sp_out[:].opt()])

        # big phase-1 pools closed: load scan A table now
        abig = ctx.enter_context(tc.tile_pool(name="abig", bufs=1))
        a_p0 = abig.tile([1, HO * HO], bf, tag="a_p0")
        nc.sync.dma_start(a_p0[:], a_dram[:].rearrange("a b -> (a b)")[None, :])

        sr = sm.tile([128, 2, 256], f32, tag="sr")
        nc.sync.dma_start(sr[:], sp_out[:, 0:256].rearrange("(h p) f -> p h f", p=128))
        pr = sm.tile([128, 2, HO], f32, tag="pr")
        nc.sync.dma_start(pr[:], sp_out[:, 256:516].rearrange("(h p) f -> p h f", p=128))

        # ---------- phase 2b: softmax rows + a_bar + base ----------
        ex = sm.tile([128, 2, 256], f32, tag="ex")
        mx = sm.tile([128, 2], f32, tag="mx")
        nm = sm.tile([128, 2], f32, tag="nm")
        rs = sm.tile([128, 2], f32, tag="rs")
        inv = sm.tile([128, 2], f32, tag="inv")
        for h in range(2):
            nc.vector.tensor_reduce(mx[:, h:h + 1], sr[:, h, :], axis=AX.X, op=ALU.max)
            nc.vector.tensor_scalar_mul(nm[:, h:h + 1], mx[:, h:h + 1], -SCALE)
            nc.scalar.activation(out=ex[:, h, :], in_=sr[:, h, :], func=AF.Exp,
                                 bias=nm[:, h:h + 1], scale=SCALE,
                                 accum_out=rs[:, h:h + 1])
            nc.vector.reciprocal(inv[:, h:h + 1], rs[:, h:h + 1])
            nc.vector.tensor_scalar_mul(ex[:, h, :], ex[:, h, :], inv[:, h:h + 1])

        with tc.tile_pool(name="psm", bufs=2, space="PSUM") as psmp:
            # a_bar as columns: abt[p, mc] = sum_m attn[m, mc*128+p] / M
            abt_ps = psmp.tile([128, 2], f32, tag="abt")
            for mc in range(2):
                for h in range(2):
                    nc.tensor.matmul(abt_ps[:, mc:mc + 1], lhsT=ex[:, h, mc * 128:(mc + 1) * 128],
                                     rhs=ones_f[:], start=(h == 0), stop=(h == 1))
            abt_sb = sm.tile([128, 2], f32, tag="abt_sb")
            nc.scalar.mul(out=abt_sb[:], in_=abt_ps[:], mul=1.0 / M)
            # base = a_bar @ P
            base_ps = psmp.tile([1, HO], f32, tag="base_ps")
            for mc in range(2):
                nc.tensor.matmul(base_ps[0:1, :], lhsT=abt_sb[:, mc:mc + 1],
                                 rhs=pr[:, mc, :], start=(mc == 0), stop=(mc == 1))
            nc.vector.tensor_add(pre_sb[:], base_ps[0:1, :], bb_s[:])

        # ---------- phase 3: 260-step scan ----------
        # v[j] = tanh(v[j-1]*A[j-1,j] + pre[j]); DVE applies v[j]'s
        # contributions to pre[j+2:] off the critical path.
        for j in range(HO):
            if j == 0:
                nc.scalar.activation(out=vrow[0:1, 0:1], in_=pre_sb[0:1, 0:1],
                                     func=AF.Tanh)
            else:
                nc.scalar.activation(out=vrow[0:1, j:j + 1], in_=vrow[0:1, j - 1:j],
                                     func=AF.Tanh,
                                     scale=band[0:1, j - 1:j, 0],
                                     bias=pre_sb[0:1, j:j + 1])
            jl = j + 2
            if jl < HO:
                nc.vector.scalar_tensor_tensor(
                    out=pre_sb[0:1, jl:HO],
                    in0=a_p0[0:1, j * HO + jl:(j + 1) * HO],
                    scalar=vrow[0:1, j:j + 1],
                    in1=pre_sb[0:1, jl:HO],
                    op0=ALU.mult, op1=ALU.add)
        nc.scalar.copy(out=y4[:], in_=vrow[0:1, HO - 4:HO])
        nc.sync.dma_start(Y[:], y4[:])

    nc.compile()
    return nc


def _get_nc():
    if "nc" not in _CACHE:
        _CACHE["nc"] = _build()
    return _CACHE["nc"]


def _make_in_maps(inputs):
    X = np.asarray(inputs["input_matrix"], np.float32)
    Wq = np.asarray(inputs["Wq"], np.float32)
    Wk = np.asarray(inputs["Wk"], np.float32)
    Wv = np.asarray(inputs["Wv"], np.float32)
    bq = np.asarray(inputs["bq"], np.float32)
    bk = np.asarray(inputs["bk"], np.float32)
    bv = np.asarray(inputs["bv"], np.float32)
    mu = np.asarray(inputs["weight_mu"], np.float32)
    sg = np.asarray(inputs["weight_sigma"], np.float32)
    ep = np.asarray(inputs["eps_w"], np.float32)
    bmu = np.asarray(inputs["bias_mu"], np.float32)
    bsg = np.asarray(inputs["bias_sigma"], np.float32)
    epb = np.asarray(inputs["eps_b"], np.float32)

    Xp = np.zeros((M, DP), np.float32)
    Xp[:, :D] = X

    muA = np.ascontiguousarray(mu[D:NTOT, D:NTOT])
    sgA = np.ascontiguousarray(sg[D:NTOT, D:NTOT])
    epA = np.ascontiguousarray(ep[D:NTOT, D:NTOT])

    in_maps = []
    for c in range(NCORES):
        st, sz = STARTS[c], SIZES[c]

        def rows2d_pad(A):
            out = np.zeros((SH, DP), np.float32)
            out[:sz, :D] = A[st:st + sz]
            return out

        def rows2d(A):
            out = np.zeros((SH, A.shape[1]), np.float32)
            out[:sz] = A[st:st + sz]
            return out

        def rows1d(a):
            out = np.zeros((SH,), np.float32)
            out[:sz] = a[st:st + sz]
            return out

        in_maps.append({
            "x": Xp,
            "wq": rows2d_pad(Wq), "wk": rows2d_pad(Wk), "wv": rows2d_pad(Wv),
            "bq": rows1d(bq), "bk": rows1d(bk), "bv": rows1d(bv),
            "mus": rows2d(mu[:, D:NTOT]),
            "sgs": rows2d(sg[:, D:NTOT]),
            "eps": rows2d(ep[:, D:NTOT]),
            "mua": muA, "sga": sgA, "epa": epA,
            "bmu": bmu, "bsg": bsg, "epb": epb,
        })
    return in_maps


def kernel(**inputs):
    from concourse.bass_utils import run_bass_kernel_spmd

    nc = _get_nc()
    in_maps = _make_in_maps(inputs)
    res = run_bass_kernel_spmd(nc, in_maps, core_ids=list(range(NCORES)))
    return np.asarray(res.results[0]["y"], np.float32).reshape(4)


# revision 11
# speedup vs baseline: 23238.9903x; 1.0307x over previous
"""Trainium2 Bass kernel for nn_BayesianNN (attention + bayesian NEAT scan).

Strategy (8 NeuronCores, SPMD, feature/row tensor-parallel):
  - Shard Wq/Wk/Wv rows (output features) across cores. Per core, W tiles
    stream in fp32, are cast to bf16 (split between ACT and DVE), PE
    transposes them into PSUM banks (8 tiles/bank), one batched copy per
    bank returns them to SBUF, and PE accumulates Q^T/K^T/V^T shards
    = W_shard @ X^T in fp32 PSUM.
  - Partial S = Q^T(shard)^T @ K^T(shard) and partial P = V @ slab
    (slab = mu+sigma*eps rows of the input->hidden block) are both
    computed in phase 1 and AllReduce'd together in one [256,516] fp32
    collective. After softmax, base = (a_bar @ P) needs just two small
    matmuls - no second collective round-trip before the bias add.
  - 260-step topological scan: one tanh per node on ACT
    (v[j] = tanh(v[j-1]*A[j-1,j] + pre[j])), with each node's
    contributions to columns >= j+2 applied off-path by a DVE rank-1
    update into the fp32 pre-activation row. v stays fp32.
"""
import sys

for _p in ("/opt/trn_rl_repo",):
    if _p not in sys.path:
        sys.path.insert(0, _p)

import numpy as np

M = 256
D = 7686
DP = 7808          # D padded to 61*128 for clean 128-chunking
NCH = DP // 128    # 61 d-chunks
HO = 260
NTOT = D + HO
NCORES = 8
SH = 1024          # padded shard rows per core
SCALE = float(1.0 / np.sqrt(np.float32(D)))

SIZES = [961] * 7 + [959]
STARTS = [sum(SIZES[:c]) for c in range(NCORES)]

# d-axis macro chunks (8 chunks of 128 each except the last with 5)
MACROS = [(i * 8, 8) for i in range(7)] + [(56, 5)]  # (chunk0, nchunks)

_CACHE = {}


def _build():
    import concourse.mybir as mybir
    import concourse.tile as tile
    from concourse import bacc
    from concourse.masks import make_identity
    from contextlib import ExitStack

    dt = mybir.dt
    f32, bf = dt.float32, dt.bfloat16
    AF = mybir.ActivationFunctionType
    ALU = mybir.AluOpType
    AX = mybir.AxisListType

    nc = bacc.Bacc(None, target_bir_lowering=False, num_devices=NCORES)

    X = nc.dram_tensor("x", [M, DP], f32, kind="ExternalInput")
    Wq = nc.dram_tensor("wq", [SH, DP], f32, kind="ExternalInput")
    Wk = nc.dram_tensor("wk", [SH, DP], f32, kind="ExternalInput")
    Wv = nc.dram_tensor("wv", [SH, DP], f32, kind="ExternalInput")
    BQ = nc.dram_tensor("bq", [SH], f32, kind="ExternalInput")
    BK = nc.dram_tensor("bk", [SH], f32, kind="ExternalInput")
    BV = nc.dram_tensor("bv", [SH], f32, kind="ExternalInput")
    MUS = nc.dram_tensor("mus", [SH, HO], f32, kind="ExternalInput")
    SGS = nc.dram_tensor("sgs", [SH, HO], f32, kind="ExternalInput")
    EPS = nc.dram_tensor("eps", [SH, HO], f32, kind="ExternalInput")
    MUA = nc.dram_tensor("mua", [HO, HO], f32, kind="ExternalInput")
    SGA = nc.dram_tensor("sga", [HO, HO], f32, kind="ExternalInput")
    EPA = nc.dram_tensor("epa", [HO, HO], f32, kind="ExternalInput")
    BMU = nc.dram_tensor("bmu", [HO], f32, kind="ExternalInput")
    BSG = nc.dram_tensor("bsg", [HO], f32, kind="ExternalInput")
    EPB = nc.dram_tensor("epb", [HO], f32, kind="ExternalInput")
    Y = nc.dram_tensor("y", [4], f32, kind="ExternalOutput")

    RG = [list(range(NCORES))]

    with tile.TileContext(nc) as tc, ExitStack() as ctx:
        const = ctx.enter_context(tc.tile_pool(name="const", bufs=1))
        sm = ctx.enter_context(tc.tile_pool(name="sm", bufs=1))
        vtp = ctx.enter_context(tc.tile_pool(name="vtp", bufs=1))
        scanp = ctx.enter_context(tc.tile_pool(name="scanp", bufs=1))
        dram = ctx.enter_context(tc.tile_pool(name="dram", bufs=1, space="DRAM"))

        idb = const.tile([128, 128], bf, tag="idb")
        make_identity(nc, idb)
        ones_f = const.tile([128, 1], f32, tag="ones_f")
        nc.vector.memset(ones_f[:], 1.0)

        vt_sb = vtp.tile([128, 8, 256], f32, tag="vt_sb")
        slab_sb = vtp.tile([128, 8, HO], f32, tag="slab_sb")

        band = scanp.tile([1, 259, 1], f32, tag="band")
        vrow = scanp.tile([1, HO], f32, tag="vrow")
        pre_sb = scanp.tile([1, HO], f32, tag="pre_sb")
        bb_s = scanp.tile([1, HO], f32, tag="bb_s")
        y4 = scanp.tile([1, 4], f32, tag="y4")

        # ---------- early prep: A block combine + scan tables + slab ----------
        with tc.tile_pool(name="aprep", bufs=1) as aprep, \
             tc.tile_pool(name="slabl", bufs=2) as slabl:
            aA = aprep.tile([128, 3, HO], f32, tag="aA")
            sA = aprep.tile([128, 3, HO], f32, tag="sA")
            eA = aprep.tile([128, 3, HO], f32, tag="eA")
            nc.sync.dma_start(aA[:, 0:2, :], MUA[0:256, :].rearrange("(c p) f -> p c f", p=128))
            nc.sync.dma_start(aA[0:4, 2, :], MUA[256:260, :])
            nc.sync.dma_start(sA[:, 0:2, :], SGA[0:256, :].rearrange("(c p) f -> p c f", p=128))
            nc.sync.dma_start(sA[0:4, 2, :], SGA[256:260, :])
            nc.sync.dma_start(eA[:, 0:2, :], EPA[0:256, :].rearrange("(c p) f -> p c f", p=128))
            nc.sync.dma_start(eA[0:4, 2, :], EPA[256:260, :])
            nc.vector.tensor_mul(sA[:, 0:2, :], sA[:, 0:2, :], eA[:, 0:2, :])
            nc.vector.tensor_add(aA[:, 0:2, :], aA[:, 0:2, :], sA[:, 0:2, :])
            nc.vector.tensor_mul(sA[0:4, 2, :], sA[0:4, 2, :], eA[0:4, 2, :])
            nc.vector.tensor_add(aA[0:4, 2, :], aA[0:4, 2, :], sA[0:4, 2, :])
            ab = aprep.tile([128, 3, HO], bf, tag="ab")
            nc.scalar.copy(out=ab[:, 0:2, :], in_=aA[:, 0:2, :])
            nc.scalar.copy(out=ab[0:4, 2, :], in_=aA[0:4, 2, :])
            a_dram = dram.tile([HO, HO], bf, tag="a_dram")
            nc.sync.dma_start(a_dram[0:256, :].rearrange("(c p) f -> p c f", p=128), ab[:, 0:2, :])
            nc.sync.dma_start(a_dram[256:260, :], ab[0:4, 2, :])
            af_dram = dram.tile([HO, HO], f32, tag="af_dram")
            nc.sync.dma_start(af_dram[0:256, :].rearrange("(c p) f -> p c f", p=128), aA[:, 0:2, :])
            nc.sync.dma_start(af_dram[256:260, :], aA[0:4, 2, :])
            # superdiagonal: band[0, k, 0] = A[k, k+1]
            af_flat = af_dram[:].rearrange("a b -> (a b)")
            nc.sync.dma_start(
                band[:], af_flat[1:1 + 259 * 261].rearrange("(k s) -> k s", s=261)[None, :, 0:1])

            bb_m = aprep.tile([1, HO], f32, tag="bb_m")
            bb_e = aprep.tile([1, HO], f32, tag="bb_e")
            nc.sync.dma_start(bb_m[:], BMU[:][None, :])
            nc.sync.dma_start(bb_s[:], BSG[:][None, :])
            nc.sync.dma_start(bb_e[:], EPB[:][None, :])
            nc.vector.tensor_mul(bb_s[:], bb_s[:], bb_e[:])
            nc.vector.tensor_add(bb_s[:], bb_s[:], bb_m[:])

            for ic in range(8):
                m_t = slabl.tile([128, HO], f32, tag="smu")
                s_t = slabl.tile([128, HO], f32, tag="ssg")
                e_t = slabl.tile([128, HO], f32, tag="sep")
                nc.sync.dma_start(m_t[:], MUS[ic * 128:(ic + 1) * 128, :])
                nc.sync.dma_start(s_t[:], SGS[ic * 128:(ic + 1) * 128, :])
                nc.sync.dma_start(e_t[:], EPS[ic * 128:(ic + 1) * 128, :])
                nc.vector.tensor_mul(s_t[:], s_t[:], e_t[:])
                nc.vector.tensor_add(slab_sb[:, ic, :], m_t[:], s_t[:])

        # ---------- phase 0+1: X^T build, then QKV shard matmuls ----------
        with tc.tile_pool(name="pa_big", bufs=1) as pab, \
             tc.tile_pool(name="wload", bufs=4) as wload, \
             tc.tile_pool(name="wcast", bufs=4) as wcast, \
             tc.tile_pool(name="wtp", bufs=4) as wtp, \
             tc.tile_pool(name="qk", bufs=4) as qk:

            bq_sb = sm.tile([128, 8], f32, tag="bq_sb")
            nc.sync.dma_start(bq_sb[:], BQ[:].rearrange("(c p) -> p c", p=128))
            bk_sb = sm.tile([128, 8], f32, tag="bk_sb")
            nc.sync.dma_start(bk_sb[:], BK[:].rearrange("(c p) -> p c", p=128))
            bv_sb = sm.tile([128, 8], f32, tag="bv_sb")
            nc.sync.dma_start(bv_sb[:], BV[:].rearrange("(c p) -> p c", p=128))

            # xt[d%128, d//128, h*128+m] = X[h*128+m, d] in bf16
            xt = pab.tile([128, NCH, 256], bf, tag="xt")

            with tc.tile_pool(name="ptr", bufs=2, space="PSUM") as ptrp, \
                 tc.tile_pool(name="pacc", bufs=2, space="PSUM") as paccp, \
                 tc.tile_pool(name="ps", bufs=1, space="PSUM") as psp, \
                 tc.tile_pool(name="pp", bufs=1, space="PSUM") as ppp, \
                 tc.tile_pool(name="xbp", bufs=1) as xbp:

                # --- X: load, cast, transpose (batched copies) ---
                xb = xbp.tile([128, 2, DP], bf, tag="xb")
                for h in range(2):
                    for im, (c0, nch) in enumerate(MACROS):
                        xl = wload.tile([128, 1024], f32, tag="wl")
                        nc.sync.dma_start(xl[:, :nch * 128],
                                          X[h * 128:(h + 1) * 128, c0 * 128:(c0 + nch) * 128])
                        eng = nc.scalar if im % 2 == 0 else nc.vector
                        if eng is nc.scalar:
                            nc.scalar.copy(out=xb[:, h, c0 * 128:(c0 + nch) * 128],
                                           in_=xl[:, :nch * 128])
                        else:
                            nc.vector.tensor_copy(out=xb[:, h, c0 * 128:(c0 + nch) * 128],
                                                  in_=xl[:, :nch * 128])
                for h in range(2):
                    for (c0, nch) in MACROS:
                        ptr = ptrp.tile([128, 8, 128], bf, tag="ptr")
                        for c in range(nch):
                            nc.tensor.transpose(ptr[:, c, :], xb[:, h, (c0 + c) * 128:(c0 + c + 1) * 128], idb[:])
                        nc.vector.tensor_copy(out=xt[:, c0:c0 + nch, h * 128:(h + 1) * 128],
                                              in_=ptr[:, 0:nch, :])

                # --- QKV streaming ---
                s_ps = psp.tile([128, 2, 256], f32, tag="s_ps")
                # P partial: 512-wide halves so each [*, h, 0:260] slice is
                # bank-aligned (1040B used of each 2KB bank)
                p_ps = ppp.tile([128, 2, 512], f32, tag="p_ps")
                for ic in range(8):
                    qt_tiles = {}
                    for mat, (wsrc, bias_sb) in (
                            ("q", (Wq, bq_sb)), ("k", (Wk, bk_sb)), ("v", (Wv, bv_sb))):
                        acc = paccp.tile([128, 256], f32, tag="pacc")
                        # software-pipelined: macro m's matmuls are emitted
                        # after macro m+1's transposes so the in-order PE
                        # never waits on the DVE copy-back of wt.
                        pending = None  # (wt, c0, nch) awaiting matmuls
                        for im, (c0, nch) in enumerate(MACROS):
                            wl = wload.tile([128, 1024], f32, tag="wl")
                            nc.sync.dma_start(wl[:, :nch * 128],
                                              wsrc[ic * 128:(ic + 1) * 128, c0 * 128:(c0 + nch) * 128])
                            wc = wcast.tile([128, 1024], bf, tag="wc")
                            if im % 3 == 0:
                                nc.vector.tensor_copy(out=wc[:, :nch * 128], in_=wl[:, :nch * 128])
                            else:
                                nc.scalar.copy(out=wc[:, :nch * 128], in_=wl[:, :nch * 128])
                            ptr = ptrp.tile([128, 8, 128], bf, tag="ptr")
                            for c in range(nch):
                                nc.tensor.transpose(ptr[:, c, :], wc[:, c * 128:(c + 1) * 128], idb[:])
                            wt = wtp.tile([128, 8, 128], bf, tag="wt")
                            nc.vector.tensor_copy(out=wt[:, 0:nch, :], in_=ptr[:, 0:nch, :])
                            if pending is not None:
                                pc0, pnch, pwt = pending
                                for c in range(pnch):
                                    nc.tensor.matmul(acc[:], lhsT=pwt[:, c, :], rhs=xt[:, pc0 + c, :],
                                                     start=(pc0 + c == 0), stop=False)
                            pending = (c0, nch, wt)
                        pc0, pnch, pwt = pending
                        for c in range(pnch):
                            nc.tensor.matmul(acc[:], lhsT=pwt[:, c, :], rhs=xt[:, pc0 + c, :],
                                             start=False, stop=(pc0 + c == NCH - 1))
                        if mat == "v":
                            nc.scalar.activation(out=vt_sb[:, ic, :], in_=acc[:],
                                                 func=AF.Identity,
                                                 bias=bias_sb[:, ic:ic + 1], scale=1.0)
                        else:
                            qt = qk.tile([128, 256], bf, tag="qt")
                            nc.scalar.activation(out=qt[:], in_=acc[:], func=AF.Identity,
                                                 bias=bias_sb[:, ic:ic + 1], scale=1.0)
                            qt_tiles[mat] = qt
                    for h in range(2):
                        nc.tensor.matmul(s_ps[:, h, :], lhsT=qt_tiles["q"][:, h * 128:(h + 1) * 128],
                                         rhs=qt_tiles["k"][:], start=(ic == 0 and h == 0),
                                         stop=(ic == 7 and h == 1))
                        nc.tensor.matmul(p_ps[:, h, 0:HO], lhsT=vt_sb[:, ic, h * 128:(h + 1) * 128],
                                         rhs=slab_sb[:, ic, :], start=(ic == 0),
                                         stop=(ic == 7), skip_group_check=True)

                # ---------- phase 2a: fused AllReduce of [S | P] ----------
                sp_in = dram.tile([M, 516], f32, tag="sp_in")
                sp_out = dram.tile([M, 516], f32, tag="sp_out", addr_space="Shared")
                s_sb = sm.tile([128, 2, 256], f32, tag="s_sb")
                nc.scalar.copy(out=s_sb[:], in_=s_ps[:])
                p_sb = sm.tile([128, 2, HO], f32, tag="p_sb")
                nc.scalar.copy(out=p_sb[:], in_=p_ps[:, :, 0:HO])
                nc.sync.dma_start(sp_in[:, 0:256].rearrange("(h p) f -> p h f", p=128), s_sb[:])
                nc.sync.dma_start(sp_in[:, 256:516].rearrange("(h p) f -> p h f", p=128), p_sb[:])
                nc.gpsimd.collective_compute("AllReduce", ALU.add, replica_groups=RG,
                                             ins=[sp_in[:].opt()], outs=[sp_out[:].opt()])

        # big phase-1 pools closed: load scan A table now
        abig = ctx.enter_context(tc.tile_pool(name="abig", bufs=1))
        a_p0 = abig.tile([1, HO * HO], bf, tag="a_p0")
        nc.sync.dma_start(a_p0[:], a_dram[:].rearrange("a b -> (a b)")[None, :])

        sr = sm.tile([128, 2, 256], f32, tag="sr")
        nc.sync.dma_start(sr[:], sp_out[:, 0:256].rearrange("(h p) f -> p h f", p=128))
        pr = sm.tile([128, 2, HO], f32, tag="pr")
        nc.sync.dma_start(pr[:], sp_out[:, 256:516].rearrange("(h p) f -> p h f", p=128))

        # ---------- phase 2b: softmax rows + a_bar + base ----------
        ex = sm.tile([128, 2, 256], f32, tag="ex")
        mx = sm.tile([128, 2], f32, tag="mx")
        nm = sm.tile([128, 2], f32, tag="nm")
        rs = sm.tile([128, 2], f32, tag="rs")
        inv = sm.tile([128, 2], f32, tag="inv")
        for h in range(2):
            nc.vector.tensor_reduce(mx[:, h:h + 1], sr[:, h, :], axis=AX.X, op=ALU.max)
            nc.vector.tensor_scalar_mul(nm[:, h:h + 1], mx[:, h:h + 1], -SCALE)
            nc.scalar.activation(out=ex[:, h, :], in_=sr[:, h, :], func=AF.Exp,
                                 bias=nm[:, h:h + 1], scale=SCALE,
                                 accum_out=rs[:, h:h + 1])
            nc.vector.reciprocal(inv[:, h:h + 1], rs[:, h:h + 1])
            nc.vector.tensor_scalar_mul(ex[:, h, :], ex[:, h, :], inv[:, h:h + 1])

        with tc.tile_pool(name="psm", bufs=2, space="PSUM") as psmp:
            # a_bar as columns: abt[p, mc] = sum_m attn[m, mc*128+p] / M
            abt_ps = psmp.tile([128, 2], f32, tag="abt")
            for mc in range(2):
                for h in range(2):
                    nc.tensor.matmul(abt_ps[:, mc:mc + 1], lhsT=ex[:, h, mc * 128:(mc + 1) * 128],
                                     rhs=ones_f[:], start=(h == 0), stop=(h == 1))
            abt_sb = sm.tile([128, 2], f32, tag="abt_sb")
            nc.scalar.mul(out=abt_sb[:], in_=abt_ps[:], mul=1.0 / M)
            # base = a_bar @ P
            base_ps = psmp.tile([1, HO], f32, tag="base_ps")
            for mc in range(2):
                nc.tensor.matmul(base_ps[0:1, :], lhsT=abt_sb[:, mc:mc + 1],
                                 rhs=pr[:, mc, :], start=(mc == 0), stop=(mc == 1))
            nc.vector.tensor_add(pre_sb[:], base_ps[0:1, :], bb_s[:])

        # ---------- phase 3: 260-step scan ----------
        # v[j] = tanh(v[j-1]*A[j-1,j] + pre[j]); DVE applies v[j]'s
        # contributions to pre[j+2:] off the critical path.
        for j in range(HO):
            if j == 0:
                nc.scalar.activation(out=vrow[0:1, 0:1], in_=pre_sb[0:1, 0:1],
                                     func=AF.Tanh)
            else:
                nc.scalar.activation(out=vrow[0:1, j:j + 1], in_=vrow[0:1, j - 1:j],
                                     func=AF.Tanh,
                                     scale=band[0:1, j - 1:j, 0],
                                     bias=pre_sb[0:1, j:j + 1])
            jl = j + 2
            if jl < HO:
                nc.vector.scalar_tensor_tensor(
                    out=pre_sb[0:1, jl:HO],
                    in0=a_p0[0:1, j * HO + jl:(j + 1) * HO],
                    scalar=vrow[0:1, j:j + 1],
                    in1=pre_sb[0:1, jl:HO],
                    op0=ALU.mult, op1=ALU.add)
        nc.scalar.copy(out=y4[:], in_=vrow[0:1, HO - 4:HO])
        nc.sync.dma_start(Y[:], y4[:])

    nc.compile()
    return nc


def _get_nc():
    if "nc" not in _CACHE:
        _CACHE["nc"] = _build()
    return _CACHE["nc"]


def _make_in_maps(inputs):
    X = np.asarray(inputs["input_matrix"], np.float32)
    Wq = np.asarray(inputs["Wq"], np.float32)
    Wk = np.asarray(inputs["Wk"], np.float32)
    Wv = np.asarray(inputs["Wv"], np.float32)
    bq = np.asarray(inputs["bq"], np.float32)
    bk = np.asarray(inputs["bk"], np.float32)
    bv = np.asarray(inputs["bv"], np.float32)
    mu = np.asarray(inputs["weight_mu"], np.float32)
    sg = np.asarray(inputs["weight_sigma"], np.float32)
    ep = np.asarray(inputs["eps_w"], np.float32)
    bmu = np.asarray(inputs["bias_mu"], np.float32)
    bsg = np.asarray(inputs["bias_sigma"], np.float32)
    epb = np.asarray(inputs["eps_b"], np.float32)

    Xp = np.zeros((M, DP), np.float32)
    Xp[:, :D] = X

    muA = np.ascontiguousarray(mu[D:NTOT, D:NTOT])
    sgA = np.ascontiguousarray(sg[D:NTOT, D:NTOT])
    epA = np.ascontiguousarray(ep[D:NTOT, D:NTOT])

    in_maps = []
    for c in range(NCORES):
        st, sz = STARTS[c], SIZES[c]

        def rows2d_pad(A):
            out = np.zeros((SH, DP), np.float32)
            out[:sz, :D] = A[st:st + sz]
            return out

        def rows2d(A):
            out = np.zeros((SH, A.shape[1]), np.float32)
            out[:sz] = A[st:st + sz]
            return out

        def rows1d(a):
            out = np.zeros((SH,), np.float32)
            out[:sz] = a[st:st + sz]
            return out

        in_maps.append({
            "x": Xp,
            "wq": rows2d_pad(Wq), "wk": rows2d_pad(Wk), "wv": rows2d_pad(Wv),
            "bq": rows1d(bq), "bk": rows1d(bk), "bv": rows1d(bv),
            "mus": rows2d(mu[:, D:NTOT]),
            "sgs": rows2d(sg[:, D:NTOT]),
            "eps": rows2d(ep[:, D:NTOT]),
            "mua": muA, "sga": sgA, "epa": epA,
            "bmu": bmu, "bsg": bsg, "epb": epb,
        })
    return in_maps


def kernel(**inputs):
    from concourse.bass_utils import run_bass_kernel_spmd

    nc = _get_nc()
    in_maps = _make_in_maps(inputs)
    res = run_bass_kernel_spmd(nc, in_maps, core_ids=list(range(NCORES)))
    return np.asarray(res.results[0]["y"], np.float32).reshape(4)
